# revision 25
# baseline (speedup 1.0000x reference)
"""Trainium2 Bass kernel for masked-softmax attention (sparse_attention).

Computes, for full inputs
    x           [H=4, N=4096, D=256] f32
    adj         [N, N] int32 (0/1)
    att_pattern [H, N, N] f32
the reference
    score = leaky_relu(att_pattern, 0.2)
    score = where(adj > 0, score, -9e15)
    ratio = softmax(score, axis=-1)
    out   = einsum('hnm,hmd->hnd', ratio, x)

Sharding: output rows (n) split across 8 cores, 512 rows each, all heads per
core. adj rows are read exactly once fleet-wide; x is replicated.

Host-side marshalling (inputs must be sliced per core on the host anyway):
att_pattern and adj are shipped fp16/uint8 and PRE-TRANSPOSED into the
[m-on-partitions, rows-free] SBUF layout the PE matmul wants for lhsT, so no
on-chip transposes are needed at all. x is shipped fp16, pre-arranged with a
ones-column appended (the ones-column makes the accumulating matmul produce
masked row-sums for free).

Per-core algorithm, per (row-block, head) tile  (atT = att^T tile, f16):
    t  = 0.2 * atT                (DVE tensor_scalar, 4x mode)
    s  = max(atT, t)              (leaky_relu; GpSimd or DVE tensor_tensor)
    e  = exp(s)                   (ACT; att ~ N(0,1) so e <= ~200, no
                                   max-subtraction needed for fp32/fp16 range)
    pT = e * adjT                 (DVE tensor_tensor; masked exp, exact zeros)
    psum[rows, 0:256] += pT.T @ x_chunk ; psum[rows, 256] += rowsum(pT)
    out_rows = psum[:, :256] * (1 / psum[:, 256])
fp16 data path, fp32 PSUM accumulation, fp32 output.
"""

import os

import numpy as np

import concourse.bass as bass
import concourse.mybir as mybir
import concourse.tile as tile
from concourse import bacc
from concourse.bass_utils import run_bass_kernel_spmd

H, N, D = 4, 4096, 256
NCORES = 8
R = N // NCORES          # rows per core = 512
RBLKS = R // 128         # 128-row blocks per core = 4
KC = N // 128            # contraction chunks = 32
DP1 = D + 1              # matmul rhs width (ones column appended)

f32 = mybir.dt.float32
f16 = mybir.dt.float16
u8 = mybir.dt.uint8
AF = mybir.ActivationFunctionType
OP = mybir.AluOpType

# Fraction of tiles whose leaky_relu runs on ACT (Prelu) instead of DVE
# (tensor_scalar + max): tile i uses ACT when i % ACT_LEAKY_MOD == 0.
# Balances the ACT exp pass against DVE's mask/normalize work.
ACT_LEAKY_MOD = int(os.environ.get("KERNEL_ACT_LEAKY_MOD", "3"))


def _emit(ctx, tc: tile.TileContext, attT: bass.AP, adjT: bass.AP,
          xb16: bass.AP, out: bass.AP):
    nc = tc.nc

    xpool = ctx.enter_context(tc.tile_pool(name="xpool", bufs=1))
    attp = ctx.enter_context(tc.tile_pool(name="attp", bufs=3))
    adjp = ctx.enter_context(tc.tile_pool(name="adjp", bufs=2))
    tpool = ctx.enter_context(tc.tile_pool(name="tpool", bufs=2))
    epool = ctx.enter_context(tc.tile_pool(name="epool", bufs=2))
    ptp = ctx.enter_context(tc.tile_pool(name="ptp", bufs=2))
    opool = ctx.enter_context(tc.tile_pool(name="opool", bufs=2))
    rpool = ctx.enter_context(tc.tile_pool(name="rpool", bufs=2))
    psum_o = ctx.enter_context(tc.tile_pool(name="psum_o", bufs=3, space="PSUM"))

    # x (pre-arranged + pre-cast on host): ONE resident tile. The loads share
    # the SP HWDGE ring with the att stream, interleaved between the early att
    # tiles in half-head (~1MB) chunks: ring FIFO then guarantees each att
    # tile isn't starved by the 8.4MB of x, and xb[h] still lands before the
    # h-th tile's matmuls need it.
    xhalves = [[xpool.tile([128, KC // 2, DP1], f16, tag=f"xb{h}_{half}",
                           name=f"xb{h}_{half}") for half in range(2)]
               for h in range(H)]

    def load_xb_half(h, half):
        ks = slice(half * (KC // 2), (half + 1) * (KC // 2))
        nc.sync.dma_start(
            xhalves[h][half],
            xb16[h].rearrange("p (k d) -> p k d", k=KC)[:, ks],
        )

    # adj tiles persist for the whole kernel (each row-block's mask is reused
    # by all four heads, which are now processed far apart). Shipped as f16
    # from the host and loaded on the same HWDGE FIFO as att — the SWDGE
    # cast path costs ~10us of cold GpSimd descriptor generation per tile.
    adjfs = [adjp.tile([128, N], f16, tag=f"adj{rb}", name=f"adjf{rb}")
             for rb in range(RBLKS)]

    # h-major tile order: only one head's x slab (2.1MB) is needed per
    # 4-tile group, so the x stream never crowds out the att stream.
    for h in range(H):
        for rb in range(RBLKS):
            rows = slice(rb * 128, (rb + 1) * 128)
            adjf = adjfs[rb]
            at = attp.tile([128, N], f16, tag="at")
            nc.sync.dma_start(at, attT[h, rb])
            if h == 0:
                nc.sync.dma_start(adjfs[rb], adjT[rb])
            if rb == 0:
                # head h's two ~1MB x chunks ride the FIFO right behind the
                # head's first att tile, ahead of this tile's matmuls.
                load_xb_half(h, 0)
                load_xb_half(h, 1)

            e = epool.tile([128, N], f16)
            if (h * RBLKS + rb) % ACT_LEAKY_MOD == 0:
                nc.scalar.activation(at, at, AF.Prelu, alpha=0.2)
                nc.scalar.activation(e, at, AF.Exp)
            else:
                t = tpool.tile([128, N], f16)
                nc.vector.tensor_scalar_mul(t, at, 0.2)
                nc.vector.tensor_tensor(t, at, t, OP.max)
                nc.scalar.activation(e, t, AF.Exp)

            pt = ptp.tile([128, N], f16)
            nc.vector.tensor_tensor(pt, e, adjf, OP.mult)

            # psum[:, :D] = p @ x[h]; psum[:, D] = rowsum(p)
            po = psum_o.tile([128, DP1], f32)
            for kk in range(KC):
                nc.tensor.matmul(
                    po,
                    lhsT=pt[:, kk * 128:(kk + 1) * 128],
                    rhs=xhalves[h][kk // (KC // 2)][:, kk % (KC // 2), :],
                    start=(kk == 0),
                    stop=(kk == KC - 1),
                )

            rec = rpool.tile([128, 1], f32)
            nc.vector.reciprocal(rec, po[:, D:DP1])
            o = opool.tile([128, D], f32)
            nc.vector.tensor_scalar_mul(o, po[:, :D], rec)
            nc.sync.dma_start(out[h, rows, :], o)


def _build():
    from contextlib import ExitStack

    nc = bacc.Bacc(None, target_bir_lowering=False)
    # attT[h, rb, p, k*128 + r] = att[h, rb*128 + r, k*128 + p]
    attT = nc.dram_tensor("attT", [H, RBLKS, 128, N], f16, kind="ExternalInput")
    # adjT[rb, p, k*128 + r] = 1.0 if adj[rb*128 + r, k*128 + p] else 0.0
    adjT = nc.dram_tensor("adjT", [RBLKS, 128, N], f16, kind="ExternalInput")
    xb16 = nc.dram_tensor("xb16", [H, 128, KC * DP1], f16, kind="ExternalInput")
    out = nc.dram_tensor("out", [H, R, D], f32, kind="ExternalOutput")
    with tile.TileContext(nc) as tc, ExitStack() as ctx:
        _emit(ctx, tc, attT.ap(), adjT.ap(), xb16.ap(), out.ap())
    nc.compile()
    return nc


_PROGRAM = None


def _get_program():
    global _PROGRAM
    if _PROGRAM is None:
        _PROGRAM = _build()
    return _PROGRAM


def _to_tiled_T(a):
    """[rows=RBLKS*128, N] -> [RBLKS, 128(p), KC*128] with
    out[rb, p, k*128 + r] = a[rb*128 + r, k*128 + p]."""
    rb = a.reshape(RBLKS, 128, KC, 128)          # [rb, r, k, p]
    return np.ascontiguousarray(rb.transpose(0, 3, 2, 1)).reshape(RBLKS, 128, N)


def make_in_maps(x, adj, att_pattern):
    x = np.asarray(x, dtype=np.float32)
    adj = np.asarray(adj)
    att16 = np.asarray(att_pattern, dtype=np.float32).astype(np.float16)
    adjm = (adj != 0).astype(np.float16)

    # [H, N, D+1] fp16 with ones column, pre-arranged to the SBUF layout
    # [H, 128, KC*(D+1)] so each head is one contiguous-per-partition DMA.
    xaug = np.empty((H, N, DP1), dtype=np.float16)
    xaug[:, :, :D] = x.astype(np.float16)
    xaug[:, :, D] = np.float16(1.0)
    xb16 = np.ascontiguousarray(
        xaug.reshape(H, KC, 128, DP1).transpose(0, 2, 1, 3).reshape(H, 128, KC * DP1)
    )

    in_maps = []
    for c in range(NCORES):
        rs = slice(c * R, (c + 1) * R)
        attT = np.stack([_to_tiled_T(att16[h, rs, :]) for h in range(H)])
        in_maps.append({
            "attT": attT,
            "adjT": _to_tiled_T(adjm[rs, :]),
            "xb16": xb16,
        })
    return in_maps


def kernel(x, adj, att_pattern, is_val=0, epoch=1, layer_position=0,
           **_unused):
    nc = _get_program()
    in_maps = make_in_maps(x, adj, att_pattern)
    res = run_bass_kernel_spmd(nc, in_maps, core_ids=list(range(NCORES)))
    return np.concatenate([r["out"] for r in res.results], axis=1)


# revision 28
# speedup vs baseline: 1.0455x; 1.0455x over previous
"""Trainium2 Bass kernel for masked-softmax attention (sparse_attention).

Computes, for full inputs
    x           [H=4, N=4096, D=256] f32
    adj         [N, N] int32 (0/1)
    att_pattern [H, N, N] f32
the reference
    score = leaky_relu(att_pattern, 0.2)
    score = where(adj > 0, score, -9e15)
    ratio = softmax(score, axis=-1)
    out   = einsum('hnm,hmd->hnd', ratio, x)

Sharding: output rows (n) split across 8 cores, 512 rows each, all heads per
core. adj rows are read exactly once fleet-wide; x is replicated.

Host-side marshalling (inputs must be sliced per core on the host anyway):
att_pattern and adj are shipped fp16/uint8 and PRE-TRANSPOSED into the
[m-on-partitions, rows-free] SBUF layout the PE matmul wants for lhsT, so no
on-chip transposes are needed at all. x is shipped fp16, pre-arranged with a
ones-column appended (the ones-column makes the accumulating matmul produce
masked row-sums for free).

Per-core algorithm, per (row-block, head) tile  (atT = att^T tile, f16):
    t  = 0.2 * atT                (DVE tensor_scalar, 4x mode)
    s  = max(atT, t)              (leaky_relu; GpSimd or DVE tensor_tensor)
    e  = exp(s)                   (ACT; att ~ N(0,1) so e <= ~200, no
                                   max-subtraction needed for fp32/fp16 range)
    pT = e * adjT                 (DVE tensor_tensor; masked exp, exact zeros)
    psum[rows, 0:256] += pT.T @ x_chunk ; psum[rows, 256] += rowsum(pT)
    out_rows = psum[:, :256] * (1 / psum[:, 256])
fp16 data path, fp32 PSUM accumulation, fp32 output.
"""

import os

import numpy as np

import concourse.bass as bass
import concourse.mybir as mybir
import concourse.tile as tile
from concourse import bacc
from concourse.bass_utils import run_bass_kernel_spmd

H, N, D = 4, 4096, 256
NCORES = 8
R = N // NCORES          # rows per core = 512
RBLKS = R // 128         # 128-row blocks per core = 4
KC = N // 128            # contraction chunks = 32
DP1 = D + 1              # matmul rhs width (ones column appended)

f32 = mybir.dt.float32
f16 = mybir.dt.float16
u8 = mybir.dt.uint8
AF = mybir.ActivationFunctionType
OP = mybir.AluOpType

# Fraction of tiles whose leaky_relu runs on ACT (Prelu) instead of DVE
# (tensor_scalar + max): tile i uses ACT when i % ACT_LEAKY_MOD == 0.
# Balances the ACT exp pass against DVE's mask/normalize work.
ACT_LEAKY_MOD = int(os.environ.get("KERNEL_ACT_LEAKY_MOD", "3"))


def _emit(ctx, tc: tile.TileContext, attT: bass.AP, adjT: bass.AP,
          xb16: bass.AP, out: bass.AP):
    nc = tc.nc

    xpool = ctx.enter_context(tc.tile_pool(name="xpool", bufs=1))
    attp = ctx.enter_context(tc.tile_pool(name="attp", bufs=2))
    adjp = ctx.enter_context(tc.tile_pool(name="adjp", bufs=1))
    tpool = ctx.enter_context(tc.tile_pool(name="tpool", bufs=2))
    epool = ctx.enter_context(tc.tile_pool(name="epool", bufs=2))
    ptp = ctx.enter_context(tc.tile_pool(name="ptp", bufs=2))
    opool = ctx.enter_context(tc.tile_pool(name="opool", bufs=2))
    rpool = ctx.enter_context(tc.tile_pool(name="rpool", bufs=2))
    psum_o = ctx.enter_context(tc.tile_pool(name="psum_o", bufs=4, space="PSUM"))

    # x (pre-arranged + pre-cast on host): one resident slab per head.
    xslabs = [xpool.tile([128, KC, DP1], f16, tag=f"xb{h}", name=f"xb{h}")
              for h in range(H)]

    # adj masks persist for the whole kernel (each row-block's mask is reused
    # by all four heads, which are processed far apart). Shipped as f16 from
    # the host (the SWDGE u8->f16 cast path costs ~10us of cold GpSimd
    # descriptor generation per DMA) and loaded in ONE 4MB DMA.
    adjall = adjp.tile([128, RBLKS, N], f16)

    # h-major tile order: only one head's x slab (2.1MB) is needed per
    # 4-tile group, so the x stream never crowds out the att stream. All
    # loads share the SP HWDGE FIFO in first-use order; att tiles are
    # fetched in 2MB row-block pairs for DMA efficiency.
    for h in range(H):
        for rbp in range(RBLKS // 2):
            at2 = attp.tile([128, 2, N], f16, tag="at")
            nc.sync.dma_start(
                at2, attT[h, rbp * 2:(rbp + 1) * 2].rearrange("rb p n -> p rb n"))
            if h == 0 and rbp == 0:
                nc.sync.dma_start(adjall, adjT.rearrange("rb p n -> p rb n"))
                nc.sync.dma_start(
                    xslabs[0], xb16[0].rearrange("p (k d) -> p k d", k=KC))
            if h < H - 1 and rbp == 1:
                nc.sync.dma_start(
                    xslabs[h + 1],
                    xb16[h + 1].rearrange("p (k d) -> p k d", k=KC))

            for sub in range(2):
                rb = rbp * 2 + sub
                rows = slice(rb * 128, (rb + 1) * 128)
                adjf = adjall[:, rb, :]
                at = at2[:, sub, :]

                e = epool.tile([128, N], f16)
                if (h * RBLKS + rb) % ACT_LEAKY_MOD == 0:
                    nc.scalar.activation(at, at, AF.Prelu, alpha=0.2)
                    nc.scalar.activation(e, at, AF.Exp)
                else:
                    t = tpool.tile([128, N], f16)
                    nc.vector.tensor_scalar_mul(t, at, 0.2)
                    nc.vector.tensor_tensor(t, at, t, OP.max)
                    nc.scalar.activation(e, t, AF.Exp)

                pt = ptp.tile([128, N], f16)
                nc.vector.tensor_tensor(pt, e, adjf, OP.mult)

                # psum[:, :D] = p @ x[h]; psum[:, D] = rowsum(p)
                po = psum_o.tile([128, DP1], f32)
                for kk in range(KC):
                    nc.tensor.matmul(
                        po,
                        lhsT=pt[:, kk * 128:(kk + 1) * 128],
                        rhs=xslabs[h][:, kk, :],
                        start=(kk == 0),
                        stop=(kk == KC - 1),
                    )

                rec = rpool.tile([128, 1], f32)
                nc.vector.reciprocal(rec, po[:, D:DP1])
                o = opool.tile([128, D], f32)
                nc.vector.tensor_scalar_mul(o, po[:, :D], rec)
                nc.sync.dma_start(out[h, rows, :], o)


def _build():
    from contextlib import ExitStack

    nc = bacc.Bacc(None, target_bir_lowering=False)
    # attT[h, rb, p, k*128 + r] = att[h, rb*128 + r, k*128 + p]
    attT = nc.dram_tensor("attT", [H, RBLKS, 128, N], f16, kind="ExternalInput")
    # adjT[rb, p, k*128 + r] = 1.0 if adj[rb*128 + r, k*128 + p] else 0.0
    adjT = nc.dram_tensor("adjT", [RBLKS, 128, N], f16, kind="ExternalInput")
    xb16 = nc.dram_tensor("xb16", [H, 128, KC * DP1], f16, kind="ExternalInput")
    out = nc.dram_tensor("out", [H, R, D], f32, kind="ExternalOutput")
    with tile.TileContext(nc) as tc, ExitStack() as ctx:
        _emit(ctx, tc, attT.ap(), adjT.ap(), xb16.ap(), out.ap())
    nc.compile()
    return nc


_PROGRAM = None


def _get_program():
    global _PROGRAM
    if _PROGRAM is None:
        _PROGRAM = _build()
    return _PROGRAM


def _to_tiled_T(a):
    """[rows=RBLKS*128, N] -> [RBLKS, 128(p), KC*128] with
    out[rb, p, k*128 + r] = a[rb*128 + r, k*128 + p]."""
    rb = a.reshape(RBLKS, 128, KC, 128)          # [rb, r, k, p]
    return np.ascontiguousarray(rb.transpose(0, 3, 2, 1)).reshape(RBLKS, 128, N)


def make_in_maps(x, adj, att_pattern):
    x = np.asarray(x, dtype=np.float32)
    adj = np.asarray(adj)
    att16 = np.asarray(att_pattern, dtype=np.float32).astype(np.float16)
    adjm = (adj != 0).astype(np.float16)

    # [H, N, D+1] fp16 with ones column, pre-arranged to the SBUF layout
    # [H, 128, KC*(D+1)] so each head is one contiguous-per-partition DMA.
    xaug = np.empty((H, N, DP1), dtype=np.float16)
    xaug[:, :, :D] = x.astype(np.float16)
    xaug[:, :, D] = np.float16(1.0)
    xb16 = np.ascontiguousarray(
        xaug.reshape(H, KC, 128, DP1).transpose(0, 2, 1, 3).reshape(H, 128, KC * DP1)
    )

    in_maps = []
    for c in range(NCORES):
        rs = slice(c * R, (c + 1) * R)
        attT = np.stack([_to_tiled_T(att16[h, rs, :]) for h in range(H)])
        in_maps.append({
            "attT": attT,
            "adjT": _to_tiled_T(adjm[rs, :]),
            "xb16": xb16,
        })
    return in_maps


def kernel(x, adj, att_pattern, is_val=0, epoch=1, layer_position=0,
           **_unused):
    nc = _get_program()
    in_maps = make_in_maps(x, adj, att_pattern)
    res = run_bass_kernel_spmd(nc, in_maps, core_ids=list(range(NCORES)))
    return np.concatenate([r["out"] for r in res.results], axis=1)
